# revision 13
# baseline (speedup 1.0000x reference)
"""Trainium2 Bass kernel for BidPrefix: per-row cumprod + 3-point gather.

Reference semantics (per row b of inputs [B, 302]):
  rates = inputs[b, :300]; bid = int(inputs[b, 300]); mp = int(inputs[b, 301])
  cpz[k] = prod(rates[:k]) (cpz[0] = 1)
  out[b] = [cpz[bid], cpz[mp+1], cpz[mp]]

Strategy: pure data parallel over 8 NeuronCores (batch sharded, padded to
8*25088 rows). Per core, tiles of 128 rows. The Vector engine runs exactly
TWO fused custom DVE ops per tile (registered at import time):

  TAPCP: accum_out = C1 + sum_k eq(Idx, C0) * cumprod(Src0)[k]

giving cpz[bid] and cpz[mp] in one 300-wide pass each (cpz[i] = cp[i-1], so
C0 = idx-1 and C1 = [idx==0] covers the empty-product edge). The third
output rides on the otherwise-idle GpSimd and Scalar engines: the mp-pass's
body output is sparse with cp[mp-1] at position mp-1, so

  cpz[mp+1] = cp[mp] = sum_s body[s] * rates[s+1]

is a gpsimd tensor_tensor multiply of the saved body with the raw tile
shifted by one column (the bid column lands where body is always zero),
followed by a Scalar-engine activation(Copy) whose accum_out performs the
sum. mp==0 rows (body all zero) are patched per group with
[mp==0] * rates[0] using a small strided DMA of column 0. All products
reproduce the reference's sequential-f32 cumprod rounding exactly.
"""

import sys

if "/opt/trn_rl_repo" not in sys.path:
    sys.path.insert(0, "/opt/trn_rl_repo")

import numpy as np

S = 300
COLS = 302
P = 128
NCORES = 8
TILES = 196
BPC = TILES * P  # 25088 rows per core
BTOT = 200000

TRACE = False
LAST_RESULTS = None

_TAP_OP = None


def _get_tap_op():
    """Register the fused cumprod+tap custom DVE op (idempotent)."""
    global _TAP_OP
    if _TAP_OP is not None:
        return _TAP_OP
    import concourse.dve_ops as dve_ops
    from concourse.dve_ops import OPS, DveOp
    from concourse.dve_spec import C0, C1, AluOp, Idx, Spec, Src0, eq, lower, scan
    from concourse.dve_uop import DveOpSpec

    name = "TAPCP_ANT"
    for op in OPS:
        if op.name == name:
            _TAP_OP = op
            return op

    def _ref(in0, in1, s0, s1, imm2):
        cp = np.cumprod(in0.astype(np.float32), axis=1, dtype=np.float32)
        n = in0.shape[1]
        k = np.asarray(s0, np.float32).reshape(-1, 1)
        mask = (np.arange(n, dtype=np.float32)[None, :] == k).astype(np.float32)
        body = mask * cp
        accum = np.asarray(s1, np.float32).reshape(-1, 1) + body.sum(
            axis=1, keepdims=True
        )
        return body, accum

    spec = Spec(
        body=eq(Idx, C0) * scan(AluOp.MULTIPLY, Src0),
        accum=AluOp.ADD,
        accum_init=C1,
        reference=_ref,
    )
    shas = {}
    for ver in ("v3", "v4"):
        u = lower(spec, ver=ver)
        shas[ver] = DveOpSpec(name=name, opcode=0, uops=u, rd1_en=False).sha(ver)
    op = DveOp(name, spec, subdim=False, uops_sha=shas)
    OPS.append(op)
    dve_ops._SUB_OPCODE_FOR_NAME[name] = (
        dve_ops._CUSTOM_DVE_ROW_BASE + len(OPS) - 1
    )
    dve_ops.CUSTOM_DVE_SPECS[name] = spec
    _TAP_OP = op
    return op


def build_nc(tiles=TILES, group=28):
    import concourse.bacc as bacc
    import concourse.mybir as mybir
    from concourse import tile

    f32 = mybir.dt.float32
    A = mybir.AluOpType
    TAP = _get_tap_op()

    bpc = tiles * P
    if tiles % group != 0:
        group = tiles
    ngroups = tiles // group

    nc = bacc.Bacc("TRN2", target_bir_lowering=False, debug=False)
    inp = nc.dram_tensor("inp", [bpc, COLS], f32, kind="ExternalInput")
    out = nc.dram_tensor("out", [bpc, 3], f32, kind="ExternalOutput")

    # row = p*tiles + t (partition-major) so group output DMAs coalesce
    vin = inp.ap().rearrange("(p t) c -> p t c", p=P)
    vout = out.ap().rearrange("(p t) k -> p t k", p=P)

    with tile.TileContext(nc) as tc:
        with (
            tc.tile_pool(name="raw", bufs=2) as rawp,
            tc.tile_pool(name="body", bufs=10) as bodyp,
            tc.tile_pool(name="junk", bufs=1) as junkp,
            tc.tile_pool(name="res", bufs=3) as resp,
            tc.tile_pool(name="grp", bufs=2) as grpp,
        ):
            junk = junkp.tile([P, S], mybir.dt.uint8)
            junkA = junkp.tile([P, S], f32, tag="junkA")

            for g in range(ngroups):
                t0 = g * group
                # batched per-group scalar prep on the scalar engine:
                # im1 = idx - 1, ind0 = relu(1 - idx) = [idx == 0]
                idx = grpp.tile([P, group, 2], f32, tag="idx")
                nc.sync.dma_start(idx, vin[:, t0 : t0 + group, S:COLS])
                idxf = idx.rearrange("p t k -> p (t k)")
                im1 = grpp.tile([P, group, 2], f32, tag="im1")
                nc.scalar.activation(
                    im1.rearrange("p t k -> p (t k)"), idxf,
                    mybir.ActivationFunctionType.Copy, bias=-1.0,
                )
                ind0 = grpp.tile([P, group, 2], f32, tag="ind0")
                nc.scalar.activation(
                    ind0.rearrange("p t k -> p (t k)"), idxf,
                    mybir.ActivationFunctionType.Relu, bias=1.0, scale=-1.0,
                )
                # rates[0] of every row in the group (for the mp==0 fix)
                r0 = grpp.tile([P, group, 1], f32, tag="r0")
                nc.sync.dma_start(r0, vin[:, t0 : t0 + group, 0:1])

                # whole group's input rows in one DMA (one sync trigger
                # instead of 28)
                graw = rawp.tile([P, group, COLS], f32, tag="raw")
                nc.sync.dma_start(graw, vin[:, t0 : t0 + group, :])

                res = resp.tile([P, group, 3], f32)
                for ti in range(group):
                    raw = graw[:, ti, :]
                    rates = raw[:, 0:S]

                    # survival = cpz[bid] = cp[bid-1] (+1 if bid==0)
                    nc.vector._custom_dve(
                        TAP,
                        out=junk,
                        in0=rates,
                        s0=im1[:, ti, 0:1],
                        s1=ind0[:, ti, 0:1],
                        accum_out=res[:, ti, 0:1],
                    )
                    # anlp_last_two = cpz[mp] = cp[mp-1] (+1 if mp==0);
                    # body kept: sparse cp[mp-1] at position mp-1
                    body = bodyp.tile([P, S], f32, tag="body")
                    nc.vector._custom_dve(
                        TAP,
                        out=body,
                        in0=rates,
                        s0=im1[:, ti, 1:2],
                        s1=ind0[:, ti, 1:2],
                        accum_out=res[:, ti, 2:3],
                    )
                    # anlp_last_one = cpz[mp+1] = sum_s body[s]*rates[s+1]:
                    # gpsimd multiplies (bid column at s+1=300 meets the
                    # always-zero body[299], so it never leaks), scalar
                    # engine's activation accumulator does the sum
                    prod = bodyp.tile([P, S], f32, tag="prod")
                    nc.gpsimd.tensor_tensor(
                        prod, body, raw[:, 1 : S + 1], A.mult
                    )
                    nc.scalar.activation(
                        junkA,
                        prod,
                        mybir.ActivationFunctionType.Copy,
                        accum_out=res[:, ti, 1:2],
                    )

                # mp==0 rows: body was all zero, add cpz[1] = rates[0]
                fix = grpp.tile([P, group], f32, tag="fix")
                nc.vector.tensor_tensor(
                    fix, ind0[:, :, 1], r0[:, :, 0], A.mult
                )
                nc.vector.tensor_tensor(
                    res[:, :, 1], res[:, :, 1], fix, A.add
                )

                nc.sync.dma_start(vout[:, t0 : t0 + group, :], res)

    nc.compile()
    return nc


_NC_CACHE = {}


def _get_nc():
    key = (TILES, 28)
    if key not in _NC_CACHE:
        _NC_CACHE[key] = build_nc()
    return _NC_CACHE[key]


def kernel(inputs):
    global LAST_RESULTS
    x = np.ascontiguousarray(np.asarray(inputs), dtype=np.float32)
    assert x.shape == (BTOT, COLS), x.shape

    npad = BPC * NCORES - BTOT
    padrows = np.zeros((npad, COLS), dtype=np.float32)
    padrows[:, :S] = 1.0
    xp = np.concatenate([x, padrows], axis=0)
    shards = xp.reshape(NCORES, BPC, COLS)

    in_maps = [{"inp": np.ascontiguousarray(shards[c])} for c in range(NCORES)]

    nc = _get_nc()
    from concourse.bass_utils import run_bass_kernel_spmd

    r = run_bass_kernel_spmd(
        nc, in_maps, core_ids=list(range(NCORES)), trace=TRACE
    )
    LAST_RESULTS = r
    y = np.concatenate([r.results[c]["out"] for c in range(NCORES)], axis=0)
    return np.ascontiguousarray(y[:BTOT]).astype(np.float32)


# revision 17
# speedup vs baseline: 1.0173x; 1.0173x over previous
"""Trainium2 Bass kernel for BidPrefix: per-row cumprod + 3-point gather.

Reference semantics (per row b of inputs [B, 302]):
  rates = inputs[b, :300]; bid = int(inputs[b, 300]); mp = int(inputs[b, 301])
  cpz[k] = prod(rates[:k]) (cpz[0] = 1)
  out[b] = [cpz[bid], cpz[mp+1], cpz[mp]]

Strategy: pure data parallel over 8 NeuronCores (batch sharded, padded to
8*25088 rows). Per core, tiles of 128 rows. The Vector engine runs exactly
TWO fused custom DVE ops per tile (registered at import time):

  TAPCP: accum_out = C1 + sum_k eq(Idx, C0) * cumprod(Src0)[k]

giving cpz[bid] and cpz[mp] in one 300-wide pass each (cpz[i] = cp[i-1], so
C0 = idx-1 and C1 = [idx==0] covers the empty-product edge). The third
output rides on the otherwise-idle GpSimd and Scalar engines: the mp-pass's
body output is sparse with cp[mp-1] at position mp-1, so

  cpz[mp+1] = cp[mp] = sum_s body[s] * rates[s+1]

is a gpsimd tensor_tensor multiply of the saved body with the raw tile
shifted by one column (the bid column lands where body is always zero),
followed by a Scalar-engine activation(Copy) whose accum_out performs the
sum. mp==0 rows (body all zero) are patched per group with
[mp==0] * rates[0] using a small strided DMA of column 0. All products
reproduce the reference's sequential-f32 cumprod rounding exactly.
"""

import sys

if "/opt/trn_rl_repo" not in sys.path:
    sys.path.insert(0, "/opt/trn_rl_repo")

import numpy as np

S = 300
COLS = 302
P = 128
NCORES = 8
TILES = 196
BPC = TILES * P  # 25088 rows per core
BTOT = 200000

TRACE = False
LAST_RESULTS = None

_TAP_OP = None


def _get_tap_op():
    """Register the fused cumprod+tap custom DVE op (idempotent)."""
    global _TAP_OP
    if _TAP_OP is not None:
        return _TAP_OP
    import concourse.dve_ops as dve_ops
    from concourse.dve_ops import OPS, DveOp
    from concourse.dve_spec import C0, C1, AluOp, Idx, Spec, Src0, eq, lower, scan
    from concourse.dve_uop import DveOpSpec

    name = "TAPCP_ANT"
    for op in OPS:
        if op.name == name:
            _TAP_OP = op
            return op

    def _ref(in0, in1, s0, s1, imm2):
        cp = np.cumprod(in0.astype(np.float32), axis=1, dtype=np.float32)
        n = in0.shape[1]
        k = np.asarray(s0, np.float32).reshape(-1, 1)
        mask = (np.arange(n, dtype=np.float32)[None, :] == k).astype(np.float32)
        body = mask * cp
        accum = np.asarray(s1, np.float32).reshape(-1, 1) + body.sum(
            axis=1, keepdims=True
        )
        return body, accum

    spec = Spec(
        body=eq(Idx, C0) * scan(AluOp.MULTIPLY, Src0),
        accum=AluOp.ADD,
        accum_init=C1,
        reference=_ref,
    )
    shas = {}
    for ver in ("v3", "v4"):
        u = lower(spec, ver=ver)
        shas[ver] = DveOpSpec(name=name, opcode=0, uops=u, rd1_en=False).sha(ver)
    op = DveOp(name, spec, subdim=False, uops_sha=shas)
    OPS.append(op)
    dve_ops._SUB_OPCODE_FOR_NAME[name] = (
        dve_ops._CUSTOM_DVE_ROW_BASE + len(OPS) - 1
    )
    dve_ops.CUSTOM_DVE_SPECS[name] = spec
    _TAP_OP = op
    return op


def build_nc(tiles=TILES, group=28):
    import concourse.bacc as bacc
    import concourse.mybir as mybir
    from concourse import tile

    f32 = mybir.dt.float32
    A = mybir.AluOpType
    TAP = _get_tap_op()

    bpc = tiles * P
    # ramped group sizes: small first groups so the Vector engine starts
    # before the bulk DMA of a full-size group lands
    groups = []
    t0 = 0
    for gsz in (4, 8, 16):
        if tiles - t0 > gsz and gsz < group:
            groups.append((t0, gsz))
            t0 += gsz
    while t0 < tiles:
        gsz = min(group, tiles - t0)
        groups.append((t0, gsz))
        t0 += gsz

    nc = bacc.Bacc("TRN2", target_bir_lowering=False, debug=False)
    inp = nc.dram_tensor("inp", [bpc, COLS], f32, kind="ExternalInput")
    out = nc.dram_tensor("out", [bpc, 3], f32, kind="ExternalOutput")

    # row = p*tiles + t (partition-major) so group output DMAs coalesce
    vin = inp.ap().rearrange("(p t) c -> p t c", p=P)
    vout = out.ap().rearrange("(p t) k -> p t k", p=P)

    with tile.TileContext(nc) as tc:
        with (
            tc.tile_pool(name="raw", bufs=2) as rawp,
            tc.tile_pool(name="body", bufs=10) as bodyp,
            tc.tile_pool(name="junk", bufs=1) as junkp,
            tc.tile_pool(name="res", bufs=3) as resp,
            tc.tile_pool(name="grp", bufs=2) as grpp,
        ):
            junk = junkp.tile([P, S], mybir.dt.uint8)
            junkA = junkp.tile([P, S], f32, tag="junkA")

            for t0, gsz in groups:
                # batched per-group scalar prep on the scalar engine:
                # im1 = idx - 1, ind0 = relu(1 - idx) = [idx == 0]
                idxT = grpp.tile([P, group, 2], f32, tag="idx")
                idx = idxT[:, :gsz, :]
                nc.sync.dma_start(idx, vin[:, t0 : t0 + gsz, S:COLS])
                idxf = idx.rearrange("p t k -> p (t k)")
                im1T = grpp.tile([P, group, 2], f32, tag="im1")
                im1 = im1T[:, :gsz, :]
                nc.scalar.activation(
                    im1.rearrange("p t k -> p (t k)"), idxf,
                    mybir.ActivationFunctionType.Copy, bias=-1.0,
                )
                ind0T = grpp.tile([P, group, 2], f32, tag="ind0")
                ind0 = ind0T[:, :gsz, :]
                nc.scalar.activation(
                    ind0.rearrange("p t k -> p (t k)"), idxf,
                    mybir.ActivationFunctionType.Relu, bias=1.0, scale=-1.0,
                )
                # rates[0] of every row in the group (for the mp==0 fix)
                r0T = grpp.tile([P, group, 1], f32, tag="r0")
                r0 = r0T[:, :gsz, :]
                nc.sync.dma_start(r0, vin[:, t0 : t0 + gsz, 0:1])

                # whole group's input rows in one DMA (one sync trigger
                # per group instead of per tile)
                grawT = rawp.tile([P, group, COLS], f32, tag="raw")
                graw = grawT[:, :gsz, :]
                nc.sync.dma_start(graw, vin[:, t0 : t0 + gsz, :])

                resT = resp.tile([P, group, 3], f32)
                res = resT[:, :gsz, :]
                for ti in range(gsz):
                    raw = graw[:, ti, :]
                    rates = raw[:, 0:S]

                    # survival = cpz[bid] = cp[bid-1] (+1 if bid==0)
                    nc.vector._custom_dve(
                        TAP,
                        out=junk,
                        in0=rates,
                        s0=im1[:, ti, 0:1],
                        s1=ind0[:, ti, 0:1],
                        accum_out=res[:, ti, 0:1],
                    )
                    # anlp_last_two = cpz[mp] = cp[mp-1] (+1 if mp==0);
                    # body kept: sparse cp[mp-1] at position mp-1
                    body = bodyp.tile([P, S], f32, tag="body")
                    nc.vector._custom_dve(
                        TAP,
                        out=body,
                        in0=rates,
                        s0=im1[:, ti, 1:2],
                        s1=ind0[:, ti, 1:2],
                        accum_out=res[:, ti, 2:3],
                    )
                    # anlp_last_one = cpz[mp+1] = sum_s body[s]*rates[s+1]:
                    # gpsimd multiplies (bid column at s+1=300 meets the
                    # always-zero body[299], so it never leaks), scalar
                    # engine's activation accumulator does the sum
                    prod = bodyp.tile([P, S], f32, tag="prod")
                    nc.gpsimd.tensor_tensor(
                        prod, body, raw[:, 1 : S + 1], A.mult
                    )
                    nc.scalar.activation(
                        junkA,
                        prod,
                        mybir.ActivationFunctionType.Copy,
                        accum_out=res[:, ti, 1:2],
                    )

                # mp==0 rows: body was all zero, add cpz[1] = rates[0]
                fixT = grpp.tile([P, group], f32, tag="fix")
                fix = fixT[:, :gsz]
                nc.vector.tensor_tensor(
                    fix, ind0[:, :, 1], r0[:, :, 0], A.mult
                )
                nc.vector.tensor_tensor(
                    res[:, :, 1], res[:, :, 1], fix, A.add
                )

                nc.sync.dma_start(vout[:, t0 : t0 + gsz, :], res)

    nc.compile()
    return nc


_NC_CACHE = {}


def _get_nc():
    key = (TILES, 28)
    if key not in _NC_CACHE:
        _NC_CACHE[key] = build_nc()
    return _NC_CACHE[key]


def kernel(inputs):
    global LAST_RESULTS
    x = np.ascontiguousarray(np.asarray(inputs), dtype=np.float32)
    assert x.shape == (BTOT, COLS), x.shape

    npad = BPC * NCORES - BTOT
    padrows = np.zeros((npad, COLS), dtype=np.float32)
    padrows[:, :S] = 1.0
    xp = np.concatenate([x, padrows], axis=0)
    shards = xp.reshape(NCORES, BPC, COLS)

    in_maps = [{"inp": np.ascontiguousarray(shards[c])} for c in range(NCORES)]

    nc = _get_nc()
    from concourse.bass_utils import run_bass_kernel_spmd

    r = run_bass_kernel_spmd(
        nc, in_maps, core_ids=list(range(NCORES)), trace=TRACE
    )
    LAST_RESULTS = r
    y = np.concatenate([r.results[c]["out"] for c in range(NCORES)], axis=0)
    return np.ascontiguousarray(y[:BTOT]).astype(np.float32)


# revision 21
# speedup vs baseline: 1.1433x; 1.1239x over previous
"""Trainium2 Bass kernel for BidPrefix: per-row cumprod + 3-point gather.

Reference semantics (per row b of inputs [B, 302]):
  rates = inputs[b, :300]; bid = int(inputs[b, 300]); mp = int(inputs[b, 301])
  cpz[k] = prod(rates[:k]) (cpz[0] = 1)
  out[b] = [cpz[bid], cpz[mp+1], cpz[mp]]

Strategy: pure data parallel over 8 NeuronCores (batch sharded, padded to
8*25088 rows). Per core, tiles of 128 rows. The Vector engine runs exactly
TWO fused custom DVE ops per tile (registered at import time):

  TAPCP: accum_out = C1 + sum_k eq(Idx, C0) * cumprod(Src0)[k]

giving cpz[bid] and cpz[mp] in one 300-wide pass each (cpz[i] = cp[i-1], so
C0 = idx-1 and C1 = [idx==0] covers the empty-product edge). The third
output rides on the otherwise-idle GpSimd and Scalar engines: the mp-pass's
body output is sparse with cp[mp-1] at position mp-1, so

  cpz[mp+1] = cp[mp] = sum_s body[s] * rates[s+1]

is a gpsimd tensor_tensor multiply of the saved body with the raw tile
shifted by one column (the bid column lands where body is always zero),
followed by a Scalar-engine activation(Copy) whose accum_out performs the
sum. mp==0 rows (body all zero) are patched per group with
[mp==0] * rates[0] using a small strided DMA of column 0. All products
reproduce the reference's sequential-f32 cumprod rounding exactly.
"""

import sys

if "/opt/trn_rl_repo" not in sys.path:
    sys.path.insert(0, "/opt/trn_rl_repo")

import numpy as np

S = 300
COLS = 302
P = 128
NCORES = 8
TILES = 196
BPC = TILES * P  # 25088 rows per core
BTOT = 200000

TRACE = False
LAST_RESULTS = None

_TAP_OP = None


def _get_tap_op():
    """Register the fused cumprod+tap custom DVE op (idempotent)."""
    global _TAP_OP
    if _TAP_OP is not None:
        return _TAP_OP
    import concourse.dve_ops as dve_ops
    from concourse.dve_ops import OPS, DveOp
    from concourse.dve_spec import C0, C1, AluOp, Idx, Spec, Src0, eq, lower, scan
    from concourse.dve_uop import DveOpSpec

    name = "TAPCP_ANT"
    for op in OPS:
        if op.name == name:
            _TAP_OP = op
            return op

    def _ref(in0, in1, s0, s1, imm2):
        cp = np.cumprod(in0.astype(np.float32), axis=1, dtype=np.float32)
        n = in0.shape[1]
        k = np.asarray(s0, np.float32).reshape(-1, 1)
        mask = (np.arange(n, dtype=np.float32)[None, :] == k).astype(np.float32)
        body = mask * cp
        accum = np.asarray(s1, np.float32).reshape(-1, 1) + body.sum(
            axis=1, keepdims=True
        )
        return body, accum

    spec = Spec(
        body=eq(Idx, C0) * scan(AluOp.MULTIPLY, Src0),
        accum=AluOp.ADD,
        accum_init=C1,
        reference=_ref,
    )
    shas = {}
    for ver in ("v3", "v4"):
        u = lower(spec, ver=ver)
        shas[ver] = DveOpSpec(name=name, opcode=0, uops=u, rd1_en=False).sha(ver)
    op = DveOp(name, spec, subdim=False, uops_sha=shas)
    OPS.append(op)
    dve_ops._SUB_OPCODE_FOR_NAME[name] = (
        dve_ops._CUSTOM_DVE_ROW_BASE + len(OPS) - 1
    )
    dve_ops.CUSTOM_DVE_SPECS[name] = spec
    _TAP_OP = op
    return op


def build_nc(tiles=TILES, group=28):
    import concourse.bacc as bacc
    import concourse.mybir as mybir
    from concourse import tile

    f32 = mybir.dt.float32
    A = mybir.AluOpType
    TAP = _get_tap_op()

    bpc = tiles * P
    # ramped group sizes: small first groups so the Vector engine starts
    # before the bulk DMA of a full-size group lands
    groups = []
    t0 = 0
    for gsz in (2, 4, 8, 16):
        if tiles - t0 > gsz and gsz < group:
            groups.append((t0, gsz))
            t0 += gsz
    while t0 < tiles:
        gsz = min(group, tiles - t0)
        groups.append((t0, gsz))
        t0 += gsz

    nc = bacc.Bacc("TRN2", target_bir_lowering=False, debug=False)
    inp = nc.dram_tensor("inp", [bpc, COLS], f32, kind="ExternalInput")
    out = nc.dram_tensor("out", [bpc, 3], f32, kind="ExternalOutput")

    # row = p*tiles + t (partition-major) so group output DMAs coalesce
    vin = inp.ap().rearrange("(p t) c -> p t c", p=P)
    vout = out.ap().rearrange("(p t) k -> p t k", p=P)

    with tile.TileContext(nc) as tc:
        with (
            tc.tile_pool(name="raw", bufs=2) as rawp,
            tc.tile_pool(name="body", bufs=10) as bodyp,
            tc.tile_pool(name="junk", bufs=1) as junkp,
            tc.tile_pool(name="res", bufs=3) as resp,
            tc.tile_pool(name="grp", bufs=2) as grpp,
        ):
            junk = junkp.tile([P, S], mybir.dt.uint8)
            junkA = junkp.tile([P, S], f32, tag="junkA")

            for t0, gsz in groups:
                # batched per-group scalar prep on the scalar engine:
                # im1 = idx - 1, ind0 = relu(1 - idx) = [idx == 0]
                idxT = grpp.tile([P, group, 2], f32, tag="idx")
                idx = idxT[:, :gsz, :]
                nc.sync.dma_start(idx, vin[:, t0 : t0 + gsz, S:COLS])
                idxf = idx.rearrange("p t k -> p (t k)")
                im1T = grpp.tile([P, group, 2], f32, tag="im1")
                im1 = im1T[:, :gsz, :]
                nc.scalar.activation(
                    im1.rearrange("p t k -> p (t k)"), idxf,
                    mybir.ActivationFunctionType.Copy, bias=-1.0,
                )
                ind0T = grpp.tile([P, group, 2], f32, tag="ind0")
                ind0 = ind0T[:, :gsz, :]
                nc.scalar.activation(
                    ind0.rearrange("p t k -> p (t k)"), idxf,
                    mybir.ActivationFunctionType.Relu, bias=1.0, scale=-1.0,
                )
                # rates[0] of every row in the group (for the mp==0 fix)
                r0T = grpp.tile([P, group, 1], f32, tag="r0")
                r0 = r0T[:, :gsz, :]
                nc.sync.dma_start(r0, vin[:, t0 : t0 + gsz, 0:1])

                # whole group's input rows in one DMA (one sync trigger
                # per group instead of per tile)
                grawT = rawp.tile([P, group, COLS], f32, tag="raw")
                graw = grawT[:, :gsz, :]
                nc.sync.dma_start(graw, vin[:, t0 : t0 + gsz, :])

                resT = resp.tile([P, group, 3], f32)
                res = resT[:, :gsz, :]
                for ti in range(gsz):
                    raw = graw[:, ti, :]
                    rates = raw[:, 0:S]

                    # survival = cpz[bid] = cp[bid-1]; bid==0 (+1) patched
                    # per group below. s1 as literal 0 keeps the scalar in
                    # the instruction immediate (no extra SBUF operand read)
                    nc.vector._custom_dve(
                        TAP,
                        out=junk,
                        in0=rates,
                        s0=im1[:, ti, 0:1],
                        s1=0.0,
                        accum_out=res[:, ti, 0:1],
                    )
                    # anlp_last_two = cpz[mp] = cp[mp-1]; mp==0 patched below;
                    # body kept: sparse cp[mp-1] at position mp-1
                    body = bodyp.tile([P, S], f32, tag="body")
                    nc.vector._custom_dve(
                        TAP,
                        out=body,
                        in0=rates,
                        s0=im1[:, ti, 1:2],
                        s1=0.0,
                        accum_out=res[:, ti, 2:3],
                    )
                    # anlp_last_one = cpz[mp+1] = sum_s body[s]*rates[s+1]:
                    # gpsimd multiplies (bid column at s+1=300 meets the
                    # always-zero body[299], so it never leaks), scalar
                    # engine's activation accumulator does the sum
                    prod = bodyp.tile([P, S], f32, tag="prod")
                    nc.gpsimd.tensor_tensor(
                        prod, body, raw[:, 1 : S + 1], A.mult
                    )
                    nc.scalar.activation(
                        junkA,
                        prod,
                        mybir.ActivationFunctionType.Copy,
                        accum_out=res[:, ti, 1:2],
                    )

                # idx==0 empty-product patches (accums were seeded with 0):
                # res0 += [bid==0]; res2 += [mp==0];
                # res1 += [mp==0]*rates[0] (body was all zero for mp==0)
                nc.gpsimd.tensor_tensor(
                    res[:, :, 0], res[:, :, 0], ind0[:, :, 0], A.add
                )
                nc.gpsimd.tensor_tensor(
                    res[:, :, 2], res[:, :, 2], ind0[:, :, 1], A.add
                )
                fixT = grpp.tile([P, group], f32, tag="fix")
                fix = fixT[:, :gsz]
                nc.gpsimd.tensor_tensor(
                    fix, ind0[:, :, 1], r0[:, :, 0], A.mult
                )
                nc.gpsimd.tensor_tensor(
                    res[:, :, 1], res[:, :, 1], fix, A.add
                )

                nc.sync.dma_start(vout[:, t0 : t0 + gsz, :], res)

    nc.compile()
    return nc


_NC_CACHE = {}


def _get_nc():
    key = (TILES, 28)
    if key not in _NC_CACHE:
        _NC_CACHE[key] = build_nc()
    return _NC_CACHE[key]


def kernel(inputs):
    global LAST_RESULTS
    x = np.ascontiguousarray(np.asarray(inputs), dtype=np.float32)
    assert x.shape == (BTOT, COLS), x.shape

    npad = BPC * NCORES - BTOT
    padrows = np.zeros((npad, COLS), dtype=np.float32)
    padrows[:, :S] = 1.0
    xp = np.concatenate([x, padrows], axis=0)
    shards = xp.reshape(NCORES, BPC, COLS)

    in_maps = [{"inp": np.ascontiguousarray(shards[c])} for c in range(NCORES)]

    nc = _get_nc()
    from concourse.bass_utils import run_bass_kernel_spmd

    r = run_bass_kernel_spmd(
        nc, in_maps, core_ids=list(range(NCORES)), trace=TRACE
    )
    LAST_RESULTS = r
    y = np.concatenate([r.results[c]["out"] for c in range(NCORES)], axis=0)
    return np.ascontiguousarray(y[:BTOT]).astype(np.float32)


# revision 25
# speedup vs baseline: 1.1464x; 1.0027x over previous
"""Trainium2 Bass kernel for BidPrefix: per-row cumprod + 3-point gather.

Reference semantics (per row b of inputs [B, 302]):
  rates = inputs[b, :300]; bid = int(inputs[b, 300]); mp = int(inputs[b, 301])
  cpz[k] = prod(rates[:k]) (cpz[0] = 1)
  out[b] = [cpz[bid], cpz[mp+1], cpz[mp]]

Strategy: pure data parallel over 8 NeuronCores (batch sharded, padded to
8*25088 rows). Per core, tiles of 128 rows. The Vector engine runs exactly
TWO fused custom DVE ops per tile (registered at import time):

  TAPCP: accum_out = C1 + sum_k eq(Idx, C0) * cumprod(Src0)[k]

giving cpz[bid] and cpz[mp] in one 300-wide pass each (cpz[i] = cp[i-1], so
C0 = idx-1 and C1 = [idx==0] covers the empty-product edge). The third
output rides on the otherwise-idle GpSimd and Scalar engines: the mp-pass's
body output is sparse with cp[mp-1] at position mp-1, so

  cpz[mp+1] = cp[mp] = sum_s body[s] * rates[s+1]

is a gpsimd tensor_tensor multiply of the saved body with the raw tile
shifted by one column (the bid column lands where body is always zero),
followed by a Scalar-engine activation(Copy) whose accum_out performs the
sum. mp==0 rows (body all zero) are patched per group with
[mp==0] * rates[0] using a small strided DMA of column 0. All products
reproduce the reference's sequential-f32 cumprod rounding exactly.
"""

import sys

if "/opt/trn_rl_repo" not in sys.path:
    sys.path.insert(0, "/opt/trn_rl_repo")

import numpy as np

S = 300
COLS = 302
P = 128
NCORES = 8
TILES = 196
BPC = TILES * P  # 25088 rows per core
BTOT = 200000

TRACE = False
LAST_RESULTS = None

_TAP_OP = None


def _get_tap_op():
    """Register the fused cumprod+tap custom DVE op (idempotent)."""
    global _TAP_OP
    if _TAP_OP is not None:
        return _TAP_OP
    import concourse.dve_ops as dve_ops
    from concourse.dve_ops import OPS, DveOp
    from concourse.dve_spec import C0, C1, AluOp, Idx, Spec, Src0, eq, lower, scan
    from concourse.dve_uop import DveOpSpec

    name = "TAPCP_ANT"
    for op in OPS:
        if op.name == name:
            _TAP_OP = op
            return op

    def _ref(in0, in1, s0, s1, imm2):
        cp = np.cumprod(in0.astype(np.float32), axis=1, dtype=np.float32)
        n = in0.shape[1]
        k = np.asarray(s0, np.float32).reshape(-1, 1)
        mask = (np.arange(n, dtype=np.float32)[None, :] == k).astype(np.float32)
        body = mask * cp
        accum = np.asarray(s1, np.float32).reshape(-1, 1) + body.sum(
            axis=1, keepdims=True
        )
        return body, accum

    spec = Spec(
        body=eq(Idx, C0) * scan(AluOp.MULTIPLY, Src0),
        accum=AluOp.ADD,
        accum_init=C1,
        reference=_ref,
    )
    shas = {}
    for ver in ("v3", "v4"):
        u = lower(spec, ver=ver)
        shas[ver] = DveOpSpec(name=name, opcode=0, uops=u, rd1_en=False).sha(ver)
    op = DveOp(name, spec, subdim=False, uops_sha=shas)
    OPS.append(op)
    dve_ops._SUB_OPCODE_FOR_NAME[name] = (
        dve_ops._CUSTOM_DVE_ROW_BASE + len(OPS) - 1
    )
    dve_ops.CUSTOM_DVE_SPECS[name] = spec
    _TAP_OP = op
    return op


def build_nc(tiles=TILES, group=28):
    import concourse.bacc as bacc
    import concourse.mybir as mybir
    from concourse import tile

    f32 = mybir.dt.float32
    A = mybir.AluOpType
    TAP = _get_tap_op()

    bpc = tiles * P
    # ramped group sizes: small first groups so the Vector engine starts
    # before the bulk DMA of a full-size group lands, and a small tail so
    # the last group's cross-engine drain chain is short
    groups = []
    t0 = 0
    for gsz in (2, 4, 8, 16):
        if tiles - t0 > gsz and gsz < group:
            groups.append((t0, gsz))
            t0 += gsz
    tail = [g for g in (8, 4) if g < group]
    ntail = sum(tail)
    while t0 < tiles - ntail:
        gsz = min(group, tiles - ntail - t0)
        groups.append((t0, gsz))
        t0 += gsz
    for gsz in tail:
        if t0 < tiles:
            gsz = min(gsz, tiles - t0)
            groups.append((t0, gsz))
            t0 += gsz

    nc = bacc.Bacc("TRN2", target_bir_lowering=False, debug=False)
    inp = nc.dram_tensor("inp", [bpc, COLS], f32, kind="ExternalInput")
    out = nc.dram_tensor("out", [bpc, 3], f32, kind="ExternalOutput")

    # row = p*tiles + t (partition-major) so group output DMAs coalesce
    vin = inp.ap().rearrange("(p t) c -> p t c", p=P)
    vout = out.ap().rearrange("(p t) k -> p t k", p=P)

    with tile.TileContext(nc) as tc:
        with (
            tc.tile_pool(name="raw", bufs=2) as rawp,
            tc.tile_pool(name="body", bufs=10) as bodyp,
            tc.tile_pool(name="junk", bufs=1) as junkp,
            tc.tile_pool(name="res", bufs=3) as resp,
            tc.tile_pool(name="grp", bufs=2) as grpp,
        ):
            junk = junkp.tile([P, S], mybir.dt.uint8)
            junkA = junkp.tile([P, S], f32, tag="junkA")

            for t0, gsz in groups:
                # whole group's input rows in one DMA (one sync trigger
                # per group instead of per tile)
                grawT = rawp.tile([P, group, COLS], f32, tag="raw")
                graw = grawT[:, :gsz, :]
                nc.sync.dma_start(graw, vin[:, t0 : t0 + gsz, :])

                # batched per-group scalar prep on the scalar engine, read
                # strided straight out of graw (no extra descriptor-heavy
                # side DMAs): im1 = idx - 1, ind0 = relu(1 - idx) = [idx==0]
                idxf = graw[:, :, S:COLS]
                im1T = grpp.tile([P, group, 2], f32, tag="im1")
                im1 = im1T[:, :gsz, :]
                nc.scalar.activation(
                    im1, idxf,
                    mybir.ActivationFunctionType.Copy, bias=-1.0,
                )
                ind0T = grpp.tile([P, group, 2], f32, tag="ind0")
                ind0 = ind0T[:, :gsz, :]
                nc.scalar.activation(
                    ind0, idxf,
                    mybir.ActivationFunctionType.Relu, bias=1.0, scale=-1.0,
                )

                resT = resp.tile([P, group, 3], f32)
                res = resT[:, :gsz, :]
                for ti in range(gsz):
                    raw = graw[:, ti, :]
                    rates = raw[:, 0:S]

                    # survival = cpz[bid] = cp[bid-1]; bid==0 (+1) patched
                    # per group below. s1 as literal 0 keeps the scalar in
                    # the instruction immediate (no extra SBUF operand read)
                    nc.vector._custom_dve(
                        TAP,
                        out=junk,
                        in0=rates,
                        s0=im1[:, ti, 0:1],
                        s1=0.0,
                        accum_out=res[:, ti, 0:1],
                    )
                    # anlp_last_two = cpz[mp] = cp[mp-1]; mp==0 patched below;
                    # body kept: sparse cp[mp-1] at position mp-1
                    body = bodyp.tile([P, S], f32, tag="body")
                    nc.vector._custom_dve(
                        TAP,
                        out=body,
                        in0=rates,
                        s0=im1[:, ti, 1:2],
                        s1=0.0,
                        accum_out=res[:, ti, 2:3],
                    )
                    # anlp_last_one = cpz[mp+1] = sum_s body[s]*rates[s+1]:
                    # gpsimd multiplies (bid column at s+1=300 meets the
                    # always-zero body[299], so it never leaks), scalar
                    # engine's activation accumulator does the sum
                    prod = bodyp.tile([P, S], f32, tag="prod")
                    nc.gpsimd.tensor_tensor(
                        prod, body, raw[:, 1 : S + 1], A.mult
                    )
                    nc.scalar.activation(
                        junkA,
                        prod,
                        mybir.ActivationFunctionType.Copy,
                        accum_out=res[:, ti, 1:2],
                    )

                # idx==0 empty-product patches (accums were seeded with 0):
                # res0 += [bid==0]; res2 += [mp==0];
                # res1 += [mp==0]*rates[0] (body was all zero for mp==0)
                nc.gpsimd.tensor_tensor(
                    res[:, :, 0], res[:, :, 0], ind0[:, :, 0], A.add
                )
                nc.gpsimd.tensor_tensor(
                    res[:, :, 2], res[:, :, 2], ind0[:, :, 1], A.add
                )
                fixT = grpp.tile([P, group], f32, tag="fix")
                fix = fixT[:, :gsz]
                nc.gpsimd.tensor_tensor(
                    fix, ind0[:, :, 1], graw[:, :, 0], A.mult
                )
                nc.gpsimd.tensor_tensor(
                    res[:, :, 1], res[:, :, 1], fix, A.add
                )

                nc.sync.dma_start(vout[:, t0 : t0 + gsz, :], res)

    nc.compile()
    return nc


_NC_CACHE = {}


def _get_nc():
    key = (TILES, 28)
    if key not in _NC_CACHE:
        _NC_CACHE[key] = build_nc()
    return _NC_CACHE[key]


def kernel(inputs):
    global LAST_RESULTS
    x = np.ascontiguousarray(np.asarray(inputs), dtype=np.float32)
    assert x.shape == (BTOT, COLS), x.shape

    npad = BPC * NCORES - BTOT
    padrows = np.zeros((npad, COLS), dtype=np.float32)
    padrows[:, :S] = 1.0
    xp = np.concatenate([x, padrows], axis=0)
    shards = xp.reshape(NCORES, BPC, COLS)

    in_maps = [{"inp": np.ascontiguousarray(shards[c])} for c in range(NCORES)]

    nc = _get_nc()
    from concourse.bass_utils import run_bass_kernel_spmd

    r = run_bass_kernel_spmd(
        nc, in_maps, core_ids=list(range(NCORES)), trace=TRACE
    )
    LAST_RESULTS = r
    y = np.concatenate([r.results[c]["out"] for c in range(NCORES)], axis=0)
    return np.ascontiguousarray(y[:BTOT]).astype(np.float32)


# revision 27
# speedup vs baseline: 1.1692x; 1.0199x over previous
"""Trainium2 Bass kernel for BidPrefix: per-row cumprod + 3-point gather.

Reference semantics (per row b of inputs [B, 302]):
  rates = inputs[b, :300]; bid = int(inputs[b, 300]); mp = int(inputs[b, 301])
  cpz[k] = prod(rates[:k]) (cpz[0] = 1)
  out[b] = [cpz[bid], cpz[mp+1], cpz[mp]]

Strategy: pure data parallel over 8 NeuronCores (batch sharded, padded to
8*25088 rows). Per core, tiles of 128 rows. The Vector engine runs exactly
TWO fused custom DVE ops per tile (registered at import time):

  TAPCP: accum_out = C1 + sum_k eq(Idx, C0) * cumprod(Src0)[k]

giving cpz[bid] and cpz[mp] in one 300-wide pass each (cpz[i] = cp[i-1], so
C0 = idx-1 and C1 = [idx==0] covers the empty-product edge). The third
output rides on the otherwise-idle GpSimd and Scalar engines: the mp-pass's
body output is sparse with cp[mp-1] at position mp-1, so

  cpz[mp+1] = cp[mp] = sum_s body[s] * rates[s+1]

is a gpsimd tensor_tensor multiply of the saved body with the raw tile
shifted by one column (the bid column lands where body is always zero),
followed by a Scalar-engine activation(Copy) whose accum_out performs the
sum. mp==0 rows (body all zero) are patched per group with
[mp==0] * rates[0] using a small strided DMA of column 0. All products
reproduce the reference's sequential-f32 cumprod rounding exactly.
"""

import sys

if "/opt/trn_rl_repo" not in sys.path:
    sys.path.insert(0, "/opt/trn_rl_repo")

import numpy as np

S = 300
COLS = 302
P = 128
NCORES = 8
TILES = 196
BPC = TILES * P  # 25088 rows per core
BTOT = 200000

TRACE = False
LAST_RESULTS = None

_TAP_OP = None


def _get_tap_op():
    """Register the fused cumprod+tap custom DVE op (idempotent)."""
    global _TAP_OP
    if _TAP_OP is not None:
        return _TAP_OP
    import concourse.dve_ops as dve_ops
    from concourse.dve_ops import OPS, DveOp
    from concourse.dve_spec import C0, C1, AluOp, Idx, Spec, Src0, eq, lower, scan
    from concourse.dve_uop import DveOpSpec

    name = "TAPCP_ANT"
    for op in OPS:
        if op.name == name:
            _TAP_OP = op
            return op

    def _ref(in0, in1, s0, s1, imm2):
        cp = np.cumprod(in0.astype(np.float32), axis=1, dtype=np.float32)
        n = in0.shape[1]
        k = np.asarray(s0, np.float32).reshape(-1, 1)
        mask = (np.arange(n, dtype=np.float32)[None, :] == k).astype(np.float32)
        body = mask * cp
        accum = np.asarray(s1, np.float32).reshape(-1, 1) + body.sum(
            axis=1, keepdims=True
        )
        return body, accum

    spec = Spec(
        body=eq(Idx, C0) * scan(AluOp.MULTIPLY, Src0),
        accum=AluOp.ADD,
        accum_init=C1,
        reference=_ref,
    )
    shas = {}
    for ver in ("v3", "v4"):
        u = lower(spec, ver=ver)
        shas[ver] = DveOpSpec(name=name, opcode=0, uops=u, rd1_en=False).sha(ver)
    op = DveOp(name, spec, subdim=False, uops_sha=shas)
    OPS.append(op)
    dve_ops._SUB_OPCODE_FOR_NAME[name] = (
        dve_ops._CUSTOM_DVE_ROW_BASE + len(OPS) - 1
    )
    dve_ops.CUSTOM_DVE_SPECS[name] = spec
    _TAP_OP = op
    return op


def build_nc(tiles=TILES, group=28):
    import concourse.bacc as bacc
    import concourse.mybir as mybir
    from concourse import tile

    f32 = mybir.dt.float32
    A = mybir.AluOpType
    TAP = _get_tap_op()

    bpc = tiles * P
    # ramped group sizes: small first groups so the Vector engine starts
    # before the bulk DMA of a full-size group lands, and a small tail so
    # the last group's cross-engine drain chain is short
    groups = []
    t0 = 0
    for gsz in (2, 4, 8, 16):
        if tiles - t0 > gsz and gsz < group:
            groups.append((t0, gsz))
            t0 += gsz
    tail = [g for g in (8, 4) if g < group]
    ntail = sum(tail)
    while t0 < tiles - ntail:
        gsz = min(group, tiles - ntail - t0)
        groups.append((t0, gsz))
        t0 += gsz
    for gsz in tail:
        if t0 < tiles:
            gsz = min(gsz, tiles - t0)
            groups.append((t0, gsz))
            t0 += gsz

    nc = bacc.Bacc("TRN2", target_bir_lowering=False, debug=False)
    inp = nc.dram_tensor("inp", [bpc, COLS], f32, kind="ExternalInput")
    out = nc.dram_tensor("out", [bpc, 3], f32, kind="ExternalOutput")

    # row = p*tiles + t (partition-major) so group output DMAs coalesce
    vin = inp.ap().rearrange("(p t) c -> p t c", p=P)
    vout = out.ap().rearrange("(p t) k -> p t k", p=P)

    with tile.TileContext(nc) as tc:
        with (
            tc.tile_pool(name="raw", bufs=3) as rawp,
            tc.tile_pool(name="body", bufs=10) as bodyp,
            tc.tile_pool(name="junk", bufs=1) as junkp,
            tc.tile_pool(name="res", bufs=3) as resp,
            tc.tile_pool(name="grp", bufs=2) as grpp,
        ):
            junk = junkp.tile([P, S], mybir.dt.uint8)
            junkA = junkp.tile([P, S], f32, tag="junkA")

            for t0, gsz in groups:
                # whole group's input rows in one DMA (one sync trigger
                # per group instead of per tile)
                grawT = rawp.tile([P, group, COLS], f32, tag="raw")
                graw = grawT[:, :gsz, :]
                half = (gsz + 1) // 2
                nc.sync.dma_start(graw[:, :half, :], vin[:, t0 : t0 + half, :])
                if half < gsz:
                    nc.sync.dma_start(
                        graw[:, half:, :], vin[:, t0 + half : t0 + gsz, :]
                    )

                # batched per-group scalar prep on the scalar engine, read
                # strided straight out of graw (no extra descriptor-heavy
                # side DMAs): im1 = idx - 1, ind0 = relu(1 - idx) = [idx==0]
                idxf = graw[:, :, S:COLS]
                im1T = grpp.tile([P, group, 2], f32, tag="im1")
                im1 = im1T[:, :gsz, :]
                nc.scalar.activation(
                    im1, idxf,
                    mybir.ActivationFunctionType.Copy, bias=-1.0,
                )
                ind0T = grpp.tile([P, group, 2], f32, tag="ind0")
                ind0 = ind0T[:, :gsz, :]
                nc.scalar.activation(
                    ind0, idxf,
                    mybir.ActivationFunctionType.Relu, bias=1.0, scale=-1.0,
                )

                resT = resp.tile([P, group, 3], f32)
                res = resT[:, :gsz, :]
                for ti in range(gsz):
                    raw = graw[:, ti, :]
                    rates = raw[:, 0:S]

                    # survival = cpz[bid] = cp[bid-1]; bid==0 (+1) patched
                    # per group below. s1 as literal 0 keeps the scalar in
                    # the instruction immediate (no extra SBUF operand read)
                    nc.vector._custom_dve(
                        TAP,
                        out=junk,
                        in0=rates,
                        s0=im1[:, ti, 0:1],
                        s1=0.0,
                        accum_out=res[:, ti, 0:1],
                    )
                    # anlp_last_two = cpz[mp] = cp[mp-1]; mp==0 patched below;
                    # body kept: sparse cp[mp-1] at position mp-1
                    body = bodyp.tile([P, S], f32, tag="body")
                    nc.vector._custom_dve(
                        TAP,
                        out=body,
                        in0=rates,
                        s0=im1[:, ti, 1:2],
                        s1=0.0,
                        accum_out=res[:, ti, 2:3],
                    )
                    # anlp_last_one = cpz[mp+1] = sum_s body[s]*rates[s+1]:
                    # gpsimd multiplies (bid column at s+1=300 meets the
                    # always-zero body[299], so it never leaks), scalar
                    # engine's activation accumulator does the sum
                    prod = bodyp.tile([P, S], f32, tag="prod")
                    nc.gpsimd.tensor_tensor(
                        prod, body, raw[:, 1 : S + 1], A.mult
                    )
                    nc.scalar.activation(
                        junkA,
                        prod,
                        mybir.ActivationFunctionType.Copy,
                        accum_out=res[:, ti, 1:2],
                    )

                # idx==0 empty-product patches (accums were seeded with 0):
                # res0 += [bid==0]; res2 += [mp==0];
                # res1 += [mp==0]*rates[0] (body was all zero for mp==0)
                nc.gpsimd.tensor_tensor(
                    res[:, :, 0], res[:, :, 0], ind0[:, :, 0], A.add
                )
                nc.gpsimd.tensor_tensor(
                    res[:, :, 2], res[:, :, 2], ind0[:, :, 1], A.add
                )
                fixT = grpp.tile([P, group], f32, tag="fix")
                fix = fixT[:, :gsz]
                nc.gpsimd.tensor_tensor(
                    fix, ind0[:, :, 1], graw[:, :, 0], A.mult
                )
                nc.gpsimd.tensor_tensor(
                    res[:, :, 1], res[:, :, 1], fix, A.add
                )

                nc.sync.dma_start(vout[:, t0 : t0 + gsz, :], res)

    nc.compile()
    return nc


_NC_CACHE = {}


def _get_nc():
    key = (TILES, 28)
    if key not in _NC_CACHE:
        _NC_CACHE[key] = build_nc()
    return _NC_CACHE[key]


def kernel(inputs):
    global LAST_RESULTS
    x = np.ascontiguousarray(np.asarray(inputs), dtype=np.float32)
    assert x.shape == (BTOT, COLS), x.shape

    npad = BPC * NCORES - BTOT
    padrows = np.zeros((npad, COLS), dtype=np.float32)
    padrows[:, :S] = 1.0
    xp = np.concatenate([x, padrows], axis=0)
    shards = xp.reshape(NCORES, BPC, COLS)

    in_maps = [{"inp": np.ascontiguousarray(shards[c])} for c in range(NCORES)]

    nc = _get_nc()
    from concourse.bass_utils import run_bass_kernel_spmd

    r = run_bass_kernel_spmd(
        nc, in_maps, core_ids=list(range(NCORES)), trace=TRACE
    )
    LAST_RESULTS = r
    y = np.concatenate([r.results[c]["out"] for c in range(NCORES)], axis=0)
    return np.ascontiguousarray(y[:BTOT]).astype(np.float32)
